# revision 13
# baseline (speedup 1.0000x reference)
"""Trainium2 Bass kernel for 16-head MHA (d_model=1024, batch 4, seq 2048).

Sharding: batch (4) x head-group (2) across 8 NeuronCores. Each core computes
one batch sample's attention for 8 of the 16 heads plus its partial output
projection; the host sums the two partial outputs per sample and adds the
bias terms.

Key structure (v3): the two heads of each head-pair keep their q/k data in
opposite 64-partition halves of SBUF, so the per-key-chunk score matmuls
(contraction = head_dim = 64) issue back-to-back into *disjoint PE row
groups* and execute concurrently (~118ns/MM vs ~426ns serial — measured).
Both heads' scores for one key chunk land in one 2-bank PSUM slab and are
exp'ed by a single N=1024 ACT call; attention runs at query-quarter (512)
granularity so PSUM fits:
  s1 slabs  2 tiles x [128,1024] f32 = 4 banks
  acc       2 tiles x [65, 512] f32  = 2 banks (heads A, B)
  proj      2 tiles x [128, 512] f32 = 2 banks
All projections (V, Q, K, and completed quarters' output projection) are
emitted as "filler" matmul bursts interleaved into the ACT-bound attention
loop so the PE never idles while the scalar engine crunches exps. Attention
starts as soon as the first K/Q chunks land (xk/xq stream in quarters on
the ACT hardware DMA queue — kept off the gpsimd software queue, which is
reserved for the normalize broadcasts). Each pair's attn@V tail + softmax
normalize is deferred past the next pair's first score chunks so the ACT
queue never drains at pair boundaries.
"""

from contextlib import ExitStack

import numpy as np

import concourse.bacc as bacc
import concourse.mybir as mybir
import concourse.tile as tile
from concourse.bass_utils import run_bass_kernel_spmd

F32 = mybir.dt.float32
F16 = mybir.dt.float16
BF16 = mybir.dt.bfloat16

D = 1024          # d_model
HD = 64           # head dim
NH_CORE = 8       # heads per core
OC = NH_CORE * HD # per-core q/k/v output dims (512)
N_CORES = 8
NI = D // 128     # contraction chunks for projections
NOC = OC // 128   # o-chunks (head pairs)
NDC = D // 128    # output-dim chunks for the final projection


def build_kernel(S=2048):
    nc = bacc.Bacc("TRN2", target_bir_lowering=False, debug=False)

    xq_d = nc.dram_tensor("xqT", (D, S), F16, kind="ExternalInput")
    xk_d = nc.dram_tensor("xkT", (D, S), F16, kind="ExternalInput")
    xv_d = nc.dram_tensor("xvT", (D, S), F16, kind="ExternalInput")
    wq_d = nc.dram_tensor("WqT", (D, OC), F16, kind="ExternalInput")
    wk_d = nc.dram_tensor("WkT", (D, OC), F16, kind="ExternalInput")
    wv_d = nc.dram_tensor("WvT", (D, OC), F16, kind="ExternalInput")
    wo_d = nc.dram_tensor("WoT", (OC, D), F16, kind="ExternalInput")
    bq_d = nc.dram_tensor("bq", (OC,), F32, kind="ExternalInput")
    bk_d = nc.dram_tensor("bk", (OC,), F32, kind="ExternalInput")
    y_d = nc.dram_tensor("yT", (D, S), F16, kind="ExternalOutput")

    NKC = S // 128            # key chunks (16)
    QT = 512                  # query-quarter size
    NQT = S // QT             # quarters (4)
    VT = min(256, S)          # xv chunk width (2 stationary tiles per chunk)
    NVG = S // VT
    NVT = VT // 128

    Exp = mybir.ActivationFunctionType.Exp
    Mult = mybir.AluOpType.mult

    with tile.TileContext(nc) as tc, ExitStack() as ctx:
        wpool = ctx.enter_context(tc.tile_pool(name="w", bufs=1))
        xpool = ctx.enter_context(tc.tile_pool(name="x", bufs=1))
        spool = ctx.enter_context(tc.tile_pool(name="seq", bufs=1))
        epool = ctx.enter_context(tc.tile_pool(name="e", bufs=5))
        evpool = ctx.enter_context(tc.tile_pool(name="ev", bufs=2))
        npool = ctx.enter_context(tc.tile_pool(name="nrm", bufs=2))
        pjpool = ctx.enter_context(tc.tile_pool(name="pj", bufs=2, space="PSUM"))
        s1pool = ctx.enter_context(tc.tile_pool(name="s1", bufs=2, space="PSUM"))
        accpool = ctx.enter_context(tc.tile_pool(name="acc", bufs=2, space="PSUM"))

        # ---- warm up the gpsimd broadcast + DVE custom recip on scratch ----
        wu = npool.tile([1, 32], F32, tag="wu", bufs=1, name="wu_rt")
        nc.vector.memset(wu, 1.0)
        nc.vector.reciprocal_approx_fast(out=wu[:, :], in_=wu[:, :])
        wub = npool.tile([64, 32], F32, tag="wub", bufs=1, name="wu_bc")
        nc.gpsimd.partition_broadcast(out_ap=wub[:, :], in_ap=wu[:, :])

        # ---- input DMAs ----
        # Everything rides the sync hardware queue in consumption-deadline
        # order (x/v chunks interleaved between slabs). Nothing goes on the
        # scalar queue — dma_start instructions there would serialize with
        # the exp stream on the ACT FIFO — and nothing on the gpsimd
        # software queue, which is reserved for the normalize broadcasts.
        bq_sb = wpool.tile([128, NOC], F32, tag="bq")
        bk_sb = wpool.tile([128, NOC], F32, tag="bk")
        wq_sb = wpool.tile([128, NI, OC], F16, tag="wq")
        wk_sb = wpool.tile([128, NI, OC], F16, tag="wk")
        wv_sb = wpool.tile([128, NI, OC], F16, tag="wv")
        wo_sb = wpool.tile([128, NOC, D], F16, tag="wo")
        xq_sb = xpool.tile([128, NI, S], F16, tag="xq")
        xk_sb = xpool.tile([128, NI, S], F16, tag="xk")

        xk_r = xk_d.ap().rearrange("(ic p) t -> p ic t", p=128)
        xq_r = xq_d.ap().rearrange("(ic p) t -> p ic t", p=128)

        xvc_tiles = {}

        def xv_fetch(g):
            tiles = [
                xpool.tile([128, VT], F16, tag="xvc", bufs=8 * NVG,
                           name=f"xvc{g}_{ic}")
                for ic in range(NI)
            ]
            for ic in range(NI):
                nc.sync.dma_start(
                    out=tiles[ic],
                    in_=xv_d.ap()[ic * 128:(ic + 1) * 128, g * VT:(g + 1) * VT],
                )
            xvc_tiles[g] = tiles

        nc.sync.dma_start(out=wk_sb, in_=wk_d.ap().rearrange("(ic p) o -> p ic o", p=128))
        nc.sync.dma_start(out=xk_sb[:, :, 0:QT], in_=xk_r[:, :, 0:QT])
        nc.sync.dma_start(out=bq_sb, in_=bq_d.ap().rearrange("(c p) -> p c", p=128))
        nc.sync.dma_start(out=bk_sb, in_=bk_d.ap().rearrange("(c p) -> p c", p=128))
        nc.sync.dma_start(out=wq_sb, in_=wq_d.ap().rearrange("(ic p) o -> p ic o", p=128))
        nc.sync.dma_start(out=xq_sb[:, :, 0:QT], in_=xq_r[:, :, 0:QT])
        nc.sync.dma_start(out=wv_sb, in_=wv_d.ap().rearrange("(ic p) o -> p ic o", p=128))
        xv_fetch(0)
        xv_fetch(1)
        nc.sync.dma_start(out=xk_sb[:, :, QT:2 * QT], in_=xk_r[:, :, QT:2 * QT])
        xv_fetch(2)
        nc.sync.dma_start(out=xk_sb[:, :, 2 * QT:3 * QT], in_=xk_r[:, :, 2 * QT:3 * QT])
        xv_fetch(3)
        nc.sync.dma_start(out=xk_sb[:, :, 3 * QT:4 * QT], in_=xk_r[:, :, 3 * QT:4 * QT])
        xv_fetch(4)
        xv_fetch(5)
        nc.sync.dma_start(out=xq_sb[:, :, QT:2 * QT], in_=xq_r[:, :, QT:2 * QT])
        xv_fetch(6)
        xv_fetch(7)
        nc.sync.dma_start(out=xq_sb[:, :, 2 * QT:3 * QT], in_=xq_r[:, :, 2 * QT:3 * QT])
        nc.sync.dma_start(out=xq_sb[:, :, 3 * QT:4 * QT], in_=xq_r[:, :, 3 * QT:4 * QT])
        nc.sync.dma_start(out=wo_sb, in_=wo_d.ap().rearrange("(oc p) d -> p oc d", p=128))

        # ---- per-sequence slabs ----
        v_sb = spool.tile([128, NKC, NH_CORE * (HD + 1)], BF16, tag="v")
        qT_sb = spool.tile([128, NOC, S], F16, tag="qT")
        kT_sb = spool.tile([128, NOC, S], F16, tag="kT")
        att_sb = [
            spool.tile([128, NOC, QT], F16, tag=f"att{p}", name=f"att{p}")
            for p in range(2)
        ]

        # ---- burst emitters ----
        def v_burst(g):
            xvc = xvc_tiles.pop(g)
            vps = [
                pjpool.tile([128, OC], F32, tag="pj", name=f"vps{g}_{j}")
                for j in range(NVT)
            ]
            for ic in range(NI):
                for j in range(NVT):
                    nc.tensor.matmul(
                        vps[j][:, :],
                        xvc[ic][:, j * 128:(j + 1) * 128],
                        wv_sb[:, ic, :],
                        start=(ic == 0), stop=(ic == NI - 1),
                    )
            for j in range(NVT):
                tci = g * NVT + j
                vv = v_sb[:, tci, :].rearrange("p (h c) -> p h c", h=NH_CORE)
                nc.vector.tensor_copy(
                    out=vv[:, :, 0:HD],
                    in_=vps[j][:, :].rearrange("p (h c) -> p h c", c=HD),
                )
                nc.vector.memset(vv[:, :, HD:HD + 1], 1.0)

        def qk_burst(which, hp, tq):
            w_sb, x_sb, b_sb, dst = (
                (wq_sb, xq_sb, bq_sb, qT_sb) if which == "q"
                else (wk_sb, xk_sb, bk_sb, kT_sb)
            )
            pp = pjpool.tile([128, QT], F32, tag="pj", name=f"pj_{which}{hp}_{tq}")
            for ic in range(NI):
                nc.tensor.matmul(
                    pp[:, :],
                    w_sb[:, ic, hp * 128:(hp + 1) * 128],
                    x_sb[:, ic, tq * QT:(tq + 1) * QT],
                    start=(ic == 0), stop=(ic == NI - 1),
                )
            nc.vector.tensor_scalar_add(
                out=dst[:, hp, tq * QT:(tq + 1) * QT],
                in0=pp[:, :],
                scalar1=b_sb[:, hp:hp + 1],
            )

        def y_burst(qt, dc):
            yps = pjpool.tile([128, QT], F32, tag="pj", name=f"yps{qt}_{dc}")
            for oc in range(NOC):
                nc.tensor.matmul(
                    yps[:, :],
                    wo_sb[:, oc, dc * 128:(dc + 1) * 128],
                    att_sb[qt % 2][:, oc, :],
                    start=(oc == 0), stop=(oc == NOC - 1),
                )
            y_sb = evpool.tile([128, QT], F16, tag="yev", name=f"yev{qt}_{dc}")
            nc.vector.tensor_copy(out=y_sb[:, :], in_=yps[:, :])
            nc.sync.dma_start(
                out=y_d.ap()[dc * 128:(dc + 1) * 128, qt * QT:(qt + 1) * QT],
                in_=y_sb[:, :],
            )

        # ---- phase 1: minimal prerequisites for attention (hp0, qt0) ----
        # dummy matmuls on scratch SBUF keep the PE HAM activity window busy
        # (and warmed to full clock) while the first x/w DMAs land
        scr = npool.tile([64, 256], F16, tag="scr", name="warm_scr")
        nc.vector.memset(scr, 0.0)
        wps = s1pool.tile([128, 2 * QT], F32, tag="s1", name="warm_ps")
        for i in range(36):
            nc.tensor.matmul(wps[:, 0:256], scr[:, 0:128], scr[:, :],
                             start=True, stop=True)
        qk_burst("k", 0, 0)
        qk_burst("q", 0, 0)

        # filler queue: V-projection groups, remaining K/Q chunks, and later
        # each finished quarter's output projection. Order tracks deadlines:
        # v(g) gates attn@V of key chunks 2g..2g+1; K(hp0,tq) gates score
        # chunk kc=4tq; K(hp) and Q(hp,0) gate pair hp's loop at slot 16*hp.
        filler = [
            lambda: v_burst(0),
            lambda: qk_burst("k", 0, 1),
            lambda: v_burst(1),
            lambda: v_burst(2),
            lambda: qk_burst("k", 0, 2),
            lambda: v_burst(3),
            lambda: v_burst(4),
            lambda: qk_burst("k", 0, 3),
            lambda: v_burst(5),
            lambda: v_burst(6),
            lambda: qk_burst("k", 1, 0),
            lambda: v_burst(7),
            lambda: qk_burst("q", 1, 0),
            lambda: qk_burst("k", 1, 1),
            lambda: qk_burst("k", 1, 2),
            lambda: qk_burst("k", 1, 3),
        ]
        for hp in range(2, NOC):
            filler.append(lambda hp=hp: qk_burst("k", hp, 0))
            filler.append(lambda hp=hp: qk_burst("q", hp, 0))
            for tq in range(1, NQT):
                filler.append(lambda hp=hp, tq=tq: qk_burst("k", hp, tq))
        for hp in range(NOC):
            filler.append(lambda hp=hp: qk_burst("q", hp, 1))

        # ---- main attention loop: quarter-outer, head-pair inner ----
        slot = 0
        tail = [None]

        def make_tail(qt, hp, accs, pend, s2):
            def run():
                for item in pend:
                    s2(*item)
                asbs = []
                for hl in range(2):
                    asb = npool.tile([HD + 1, QT], F32, tag="accsb", bufs=2,
                                     name=f"asb{qt}_{hp}_{hl}")
                    nc.vector.tensor_copy(out=asb[:, :], in_=accs[hl][:, :])
                    asbs.append(asb)
                for hl in range(2):
                    off = hl * 64
                    asb = asbs[hl]
                    rt = npool.tile([1, QT], F32, tag="rtmp", bufs=1,
                                    name=f"rt{qt}_{hp}_{hl}")
                    nc.vector.tensor_copy(out=rt[:, :], in_=asb[64:65, :])
                    nc.vector.reciprocal_approx_fast(out=rt[:, :], in_=rt[:, :])
                    bc = npool.tile([64, QT], F32, tag="bcast", bufs=1,
                                    name=f"bc{qt}_{hp}_{hl}")
                    nc.gpsimd.partition_broadcast(out_ap=bc[:, :], in_ap=rt[:, :])
                    nc.vector.tensor_tensor(
                        out=att_sb[qt % 2][off:off + 64, hp, :],
                        in0=asb[0:64, :],
                        in1=bc[:, :],
                        op=Mult,
                    )
            return run

        for qt in range(NQT):
            if qt > 0:
                for hp in range(NOC):
                    if qt + 1 < NQT:
                        filler.append(lambda hp=hp, t=qt + 1: qk_burst("q", hp, t))
                for dc in range(NDC):
                    filler.append(lambda q=qt - 1, dc=dc: y_burst(q, dc))
            for hp in range(NOC):
                accs = [
                    accpool.tile([HD + 1, QT], F32, tag="acc", name=f"acc{qt}_{hp}_{hl}")
                    for hl in range(2)
                ]
                pend = []

                def s2(e, kc, hp=hp, accs=accs):
                    for hl in range(2):
                        h = 2 * hp + hl
                        nc.tensor.matmul(
                            accs[hl][:, :],
                            v_sb[:, kc, h * (HD + 1):(h + 1) * (HD + 1)],
                            e[:, hl * QT:(hl + 1) * QT],
                            start=(kc == 0), stop=(kc == NKC - 1),
                        )

                for kc in range(NKC):
                    s1 = s1pool.tile([128, 2 * QT], F32, tag="s1",
                                     name=f"s1_{qt}_{hp}_{kc}")
                    for hl in range(2):
                        off = hl * 64
                        nc.tensor.matmul(
                            s1[:, hl * QT:(hl + 1) * QT],
                            kT_sb[off:off + 64, hp, kc * 128:(kc + 1) * 128],
                            qT_sb[off:off + 64, hp, qt * QT:(qt + 1) * QT],
                            start=True, stop=True,
                        )
                    e = epool.tile([128, 2 * QT], BF16, tag="e",
                                   name=f"e{qt}_{hp}_{kc}")
                    nc.scalar.activation(out=e[:, :], in_=s1[:, :], func=Exp)
                    pend.append((e, kc))
                    if kc == 1 and tail[0] is not None:
                        # previous pair's attn@V tail + normalize, deferred
                        # past this pair's first scores so ACT never drains
                        tail[0]()
                        tail[0] = None
                    if len(pend) > 2:
                        s2(*pend.pop(0))
                    slot += 1
                    if filler and slot % (1 if slot < 16 else 3) == 0:
                        filler.pop(0)()
                tail[0] = make_tail(qt, hp, accs, pend, s2)

        tail[0]()
        while filler:
            filler.pop(0)()
        for dc in range(NDC):
            y_burst(NQT - 1, dc)

    nc.compile()
    return nc


def make_in_maps(query, key, value, Wq, bq, Wk, bk, Wv, bv, Wo, bo):
    """Shard + lay out full inputs for the 8 cores: core = 2*n + g."""
    f16 = np.float16
    N = query.shape[0]
    per_g = {}
    for g in range(2):
        osl = slice(g * OC, (g + 1) * OC)
        per_g[g] = dict(
            WqT=np.ascontiguousarray(Wq[osl, :].T).astype(f16),
            WkT=np.ascontiguousarray(Wk[osl, :].T).astype(f16),
            WvT=np.ascontiguousarray(Wv[osl, :].T).astype(f16),
            WoT=np.ascontiguousarray(Wo[:, osl].T).astype(f16),
            bq=np.ascontiguousarray(bq[osl]).astype(np.float32),
            bk=np.ascontiguousarray(bk[osl]).astype(np.float32),
        )
    in_maps = []
    for n in range(N):
        xqT = np.ascontiguousarray(query[n].T).astype(f16)
        xkT = np.ascontiguousarray(key[n].T).astype(f16)
        xvT = np.ascontiguousarray(value[n].T).astype(f16)
        for g in range(2):
            m = dict(xqT=xqT, xkT=xkT, xvT=xvT)
            m.update(per_g[g])
            in_maps.append(m)
    return in_maps


_BUILT = None


def _get_built():
    global _BUILT
    if _BUILT is None:
        _BUILT = build_kernel(2048)
    return _BUILT


def kernel(query, key, value, Wq, bq, Wk, bk, Wv, bv, Wo, bo, _results=None):
    query = np.asarray(query, np.float32)
    key = np.asarray(key, np.float32)
    value = np.asarray(value, np.float32)
    Wq, bq = np.asarray(Wq, np.float32), np.asarray(bq, np.float32)
    Wk, bk = np.asarray(Wk, np.float32), np.asarray(bk, np.float32)
    Wv, bv = np.asarray(Wv, np.float32), np.asarray(bv, np.float32)
    Wo, bo = np.asarray(Wo, np.float32), np.asarray(bo, np.float32)

    N, S, _ = query.shape
    if _results is None:
        nc = _get_built()
        in_maps = make_in_maps(query, key, value, Wq, bq, Wk, bk, Wv, bv, Wo, bo)
        res = run_bass_kernel_spmd(nc, in_maps, list(range(N_CORES)))
        _results = res.results

    const = bv @ Wo.T + bo  # host-folded bias terms
    out = np.empty((N, S, D), np.float32)
    for n in range(N):
        yT = _results[2 * n]["yT"].astype(np.float32) + \
            _results[2 * n + 1]["yT"].astype(np.float32)
        out[n] = yT.T + const
    return out
